# revision 39
# baseline (speedup 1.0000x reference)
"""Distributed NT-Xent contrastive loss kernel for Trainium2 (8 NeuronCores).

Strategy (data-parallel over batch, all-gather of projections):
  - Each core owns a 256-row shard of the 2048-sequence batch.
  - On device: gather last-valid-timestep rows (indirect DMA), project
    D=512 -> P=256 (fp32 matmuls, bias folded in as an extra K=1
    contraction row), row-normalize (cosine) with the 1/temperature
    scale folded in as sqrt(20) on each operand, transpose to [P, n]
    layout, AllGather the projections across the 8 cores (split into
    two 128-dim halves so the second collective overlaps the first
    half's similarity matmuls).
  - Each core computes its 256 rows of the full 2048x2048 logit matrix
    G = z_shard @ z_full^T. The own-diagonal mask (-1e30) is folded
    into each PSUM accumulation group as one extra bf16 matmul:
    (-1e30 * I) @ onehot, where onehot[i, j] = (j == diag_col_i) is
    built once per row-tile from an iota compare. Then a fused
    exp(x - 20) + row-sum runs on the scalar engine (20 is the
    self-similarity upper bound, so no max pass is needed), then
    log -> per-row logsumexp - 20.
  - The label logits (diag of sim12) are the elementwise product of the
    core's shard with its partner core's shard (indirect-gathered from
    the AllGather output), reduced over P via a ones-matmul.
  - Each core outputs [sum_rows log(S), sum_rows label]; the host sums
    the 8 partial pairs: loss = (sum A - sum L)/2048 + 20.
"""

import os
import sys

import ml_dtypes
import numpy as np

for _p in ("/root/.axon_site", "/root/.axon_site/_ro/trn_rl_repo",
           "/root/.axon_site/_ro/pypackages", "/opt/trn_rl_repo"):
    if os.path.isdir(_p) and _p not in sys.path:
        sys.path.append(_p)

import concourse.bacc as bacc
import concourse.bass as bass
import concourse.mybir as mybir
import concourse.tile as tile
from concourse.bass import IndirectOffsetOnAxis
from concourse.bass_utils import run_bass_kernel_spmd
from concourse.masks import make_identity

F32 = mybir.dt.float32
BF16 = mybir.dt.bfloat16
I32 = mybir.dt.int32
I16 = mybir.dt.int16

NCORES = 8
B2, S, D, P = 2048, 64, 512, 256
B = B2 // 2
SH = B2 // NCORES          # 256 rows per core
NT = SH // 128             # row tiles per core (2)
KT = P // 128              # contraction tiles over projection dim (2)
DT = D // 128              # tiles over representation dim (4)
NB = B2 // 512             # 512-wide column blocks of G (4)
INV_T = 20.0               # 1 / temperature
NEG_BIG = -1.0e30
F16 = mybir.dt.float16
# All-gather the projections in fp16: half the collective bytes and 4x
# faster similarity matmuls. Measured loss error vs the fp32 reference is
# 1.6e-06 relative (the fp16 rounding averages out over the 2048-row mean);
# set NTX_AG16=0 to fall back to full fp32 (6.3e-07).
AG16 = os.environ.get("NTX_AG16", "1") == "1"
# Measured same-process: psG=6 banks + hoisted log(S) is ~9 us SLOWER (the
# hoisted Ln between exp batches forces an ACT Exp-table reload, and a fully
# packed PSUM removes scheduler slack). Keep 4 banks and deferred Ln.
PSG_BUFS = 4
LN_HOIST = False


def build_nc(repeat=1):
    nc = bacc.Bacc("TRN2", target_bir_lowering=False, debug=False,
                   num_devices=NCORES, num_swdge_queues=2)

    reps = nc.dram_tensor("reps", [SH * S, D], F32, kind="ExternalInput")
    proj_w = nc.dram_tensor("proj_w", [P, D], F32, kind="ExternalInput")
    proj_b = nc.dram_tensor("proj_b", [P], F32, kind="ExternalInput")
    # meta int32 [128, 8]: cols 0,1 = lengths (t=0,1); 2,3 = diag col
    # (f32 bits, t=0,1); 4 = partner row index within a 128-row AG block.
    meta = nc.dram_tensor("meta", [128, 8], I32, kind="ExternalInput")
    # gmeta int32 [128, 32]: dma_gather index metadata in the 16-partition
    # wrap (idx i at [i%16, i//16], replicated over the 8 partition groups).
    # cols 0:16 = sequence base 64*n, cols 16:32 = lengths[n], n = 16*s+p%16.
    gmeta = nc.dram_tensor("gmeta", [128, 32], I32, kind="ExternalInput")
    # own-diagonal one-hot rows (bf16), used as the mask matmul rhs:
    # onehots[p, t, j] = (j == global row index of local row t*128+p).
    onehots = nc.dram_tensor("onehots", [128, NT * B2], BF16,
                             kind="ExternalInput")
    out = nc.dram_tensor("out", [1, 2], F32, kind="ExternalOutput")

    for _ in range(repeat):
        with tile.TileContext(nc) as tc:
            _body(tc, nc, reps, proj_w, proj_b, meta, gmeta, onehots, out)
    nc.compile()
    return nc


def _body(tc, nc, reps, proj_w, proj_b, meta, gmeta, onehots, out,
          mode="full"):
    with (
        tc.tile_pool(name="const", bufs=1) as cp,
        tc.tile_pool(name="work", bufs=1) as wp,
        tc.tile_pool(name="scratch", bufs=3) as sp,
        tc.tile_pool(name="dram", bufs=1, space="DRAM") as dp,
    ):
        # ---- constants / metadata ----------------------------------
        ident = cp.tile([128, 128], F32)
        make_identity(nc, ident)
        idb = cp.tile([128, 128], BF16)     # -1e30 * I for the mask matmul
        nc.vector.tensor_scalar_mul(idb[:], ident[:], NEG_BIG)
        ones_k1 = cp.tile([1, 128], F32)
        nc.gpsimd.memset(ones_k1[:], 1.0)
        ones_col = cp.tile([128, 1], F32)
        nc.gpsimd.memset(ones_col[:], 1.0)
        neg_shift = cp.tile([128, 1], F32)
        nc.gpsimd.memset(neg_shift[:], -INV_T)

        meta_sb = cp.tile([128, 8], I32)
        nc.sync.dma_start(out=meta_sb[:], in_=meta[:, :])
        b_sb = cp.tile([1, P], F32)
        nc.sync.dma_start(out=b_sb[:], in_=proj_b[None, :])
        w_sb = []
        for h in range(2):
            w_h = cp.tile([128, D], F32, name=f"w_{h}", tag=f"w_{h}")
            nc.sync.dma_start(out=w_h[:], in_=proj_w[h * 128:(h + 1) * 128, :])
            w_sb.append(w_h)

        if mode == "empty":
            res = wp.tile([1, 2], F32, name="res", tag="res")
            nc.vector.tensor_copy(res[:], meta_sb[:1, 0:2].bitcast(F32))
            nc.sync.dma_start(out=out[:, :], in_=res[:])
            return

        # own-diagonal one-hots (bf16), used as the mask matmul rhs
        oh_sb = cp.tile([128, NT, B2], BF16)
        nc.sync.dma_start(out=oh_sb[:],
                          in_=onehots.rearrange("p (t j) -> p t j", t=NT))
        onehot = [oh_sb[:, t, :] for t in range(NT)]

        if mode == "consts":
            res = wp.tile([1, 2], F32, name="res", tag="res")
            nc.vector.tensor_tensor(res[:], onehot[0][:1, 0:2],
                                    onehot[1][:1, 0:2],
                                    op=mybir.AluOpType.add)
            nc.sync.dma_start(out=out[:, :], in_=res[:])
            return

        # ---- gather last-valid-timestep rows (one dma_gather per
        # 128-row half, on separate SWDGE queues) ---------------------
        gmeta_sb = cp.tile([128, 32], I32)
        nc.sync.dma_start(out=gmeta_sb[:], in_=gmeta[:, :])
        traj = []
        for t in range(NT):
            base = gmeta_sb[:, t * 16:t * 16 + 8]
            lens = gmeta_sb[:, t * 16 + 8:t * 16 + 16]
            idx_t = wp.tile([128, 8], I32, name=f"idx_{t}", tag=f"idx_{t}")
            nc.vector.tensor_scalar(idx_t[:], lens, -1, 0,
                                    op0=mybir.AluOpType.add,
                                    op1=mybir.AluOpType.max)
            nc.vector.tensor_scalar(idx_t[:], idx_t[:], S - 1, None,
                                    op0=mybir.AluOpType.min)
            nc.vector.tensor_tensor(idx_t[:], idx_t[:], base,
                                    op=mybir.AluOpType.add)
            idxs16 = wp.tile([128, 8], I16, name=f"idxs16_{t}",
                             tag=f"idxs16_{t}")
            nc.vector.tensor_copy(idxs16[:], idx_t[:])
            gout_t = wp.tile([128, 1, D], F32, name=f"gout_{t}",
                             tag=f"gout_{t}")
            nc.gpsimd.dma_gather(
                out_ap=gout_t[:], in_ap=reps[:, :], idxs_ap=idxs16[:],
                num_idxs=128, num_idxs_reg=128, elem_size=D, queue_num=t)
            traj.append(gout_t[:, 0, :])

        if mode == "gather":
            res = wp.tile([1, 2], F32, name="res", tag="res")
            nc.vector.tensor_tensor(res[:], traj[0][:1, 0:2],
                                    traj[1][:1, 0:2],
                                    op=mybir.AluOpType.add)
            nc.sync.dma_start(out=out[:, :], in_=res[:])
            return

        zt = [[None] * KT for _ in range(NT)]   # zt[t][k]: [128p, 128n]
        ag_out = [None] * NT
        ag_in = [None] * NT

        def emit_ag(t):
            ag_in[t] = dp.tile([P, 128], F16 if AG16 else F32,
                               name=f"ag_in_{t}")
            for k in range(KT):
                nc.sync.dma_start(out=ag_in[t][k * 128:(k + 1) * 128, :],
                                  in_=zt[t][k][:])
            ag_out[t] = dp.tile(
                [NCORES * P, 128], F16 if AG16 else F32, name=f"ag_out_{t}",
                addr_space=("Local" if mode == "noag" else "Shared"))
            if mode == "noag":
                for r in range(NCORES):
                    nc.sync.dma_start(out=ag_out[t][r * P:(r + 1) * P, :],
                                      in_=ag_in[t][:, :])
            else:
                nc.gpsimd.collective_compute(
                    "AllGather", mybir.AluOpType.bypass,
                    replica_groups=[list(range(NCORES))],
                    ins=[ag_in[t].opt()], outs=[ag_out[t].opt()])

        rhs = [[[None] * 2 for _ in range(KT)] for _ in range(NT)]
        partner = [[None] * KT for _ in range(NT)]

        def emit_rhs_and_partner(h):
            ag_v = ag_out[h].rearrange("(r q) j -> q r j", r=NCORES)
            for k in range(KT):
                for q in range(2):
                    rt = wp.tile([128, 4, 128], F16 if AG16 else F32,
                                 name=f"rhs_{h}_{k}_{q}",
                                 tag=f"rhs_{h}_{k}_{q}")
                    nc.sync.dma_start(
                        out=rt[:],
                        in_=ag_v[k * 128:(k + 1) * 128, 4 * q:4 * q + 4, :])
                    rhs[h][k][q] = rt
            # partner shard columns for the labels of own row-tile h
            for k in range(KT):
                pt = wp.tile([128, 128], F16 if AG16 else F32,
                             name=f"part_{h}_{k}", tag=f"part_{h}_{k}")
                pc = 5 + k
                nc.gpsimd.indirect_dma_start(
                    out=pt[:], out_offset=None, in_=ag_out[h][:, :],
                    in_offset=IndirectOffsetOnAxis(ap=meta_sb[:, pc:pc + 1],
                                                   axis=0))
                partner[h][k] = pt

        with tc.tile_pool(name="psA", bufs=1, space="PSUM") as psA:
            # w -> wT [d, p] (shared by both halves)
            wT = [cp.tile([128, P], F16 if AG16 else F32, name=f"wT_{d}",
                     tag=f"wT_{d}") for d in range(DT)]
            for d in range(DT):
                tpw = psA.tile([128, 256], F32, tag="tpw", bufs=2)
                for h in range(2):
                    nc.tensor.transpose(tpw[:, h * 128:(h + 1) * 128],
                                        w_sb[h][:, d * 128:(d + 1) * 128],
                                        ident[:])
                nc.vector.tensor_copy(wT[d][:], tpw[:])

            trajT = [[None] * DT for _ in range(NT)]

            def emit_traj_transposes(t):
                for d in range(DT):
                    tp1 = psA.tile([128, 128], F32, tag="tp1", bufs=3)
                    nc.tensor.transpose(tp1[:],
                                        traj[t][:, d * 128:(d + 1) * 128],
                                        ident[:])
                    tt = cp.tile([128, 128], F16 if AG16 else F32,
                                 name=f"trajT_{t}_{d}",
                                 tag=f"trajT_{t}_{d}")
                    nc.vector.tensor_copy(tt[:], tp1[:])
                    trajT[t][d] = tt

            def emit_proj_norm(t):
                proj_ps = psA.tile([128, P], F32, tag="proj", bufs=2)
                for d in range(DT):
                    nc.tensor.matmul(proj_ps[:], lhsT=trajT[t][d][:],
                                     rhs=wT[d][:], start=(d == 0), stop=False)
                nc.tensor.matmul(proj_ps[:], lhsT=ones_k1[:], rhs=b_sb[:],
                                 start=False, stop=True)
                proj_sb = wp.tile([128, P], F32, name=f"proj_{t}",
                                  tag=f"proj_{t}")
                nc.vector.tensor_copy(proj_sb[:], proj_ps[:])
                sq_scr = sp.tile([128, P], F32, tag="sq")
                nsq = wp.tile([128, 1], F32, name=f"nsq_{t}", tag=f"nsq_{t}")
                nc.vector.scalar_tensor_tensor(
                    sq_scr[:], in0=proj_sb[:], scalar=1.0, in1=proj_sb[:],
                    op0=mybir.AluOpType.mult, op1=mybir.AluOpType.mult,
                    accum_out=nsq[:])
                # inv = sqrt(INV_T / max(nsq, 1e-16))
                nc.vector.tensor_scalar_max(nsq[:], nsq[:], 1e-16)
                rcp = wp.tile([128, 1], F32, name=f"rcp_{t}", tag=f"rcp_{t}")
                nc.vector.reciprocal(rcp[:], nsq[:])
                nc.vector.tensor_scalar_mul(rcp[:], rcp[:], INV_T)
                inv = wp.tile([128, 1], F32, name=f"inv_{t}", tag=f"inv_{t}")
                nc.scalar.activation(inv[:], rcp[:],
                                     mybir.ActivationFunctionType.Sqrt)
                z_t = wp.tile([128, P], F32, name=f"z_{t}", tag=f"z_{t}")
                nc.vector.tensor_scalar_mul(z_t[:], proj_sb[:], inv[:, :1])
                return z_t

            def emit_zt(t, z_t):
                for k in range(KT):
                    tp1 = psA.tile([128, 128], F32, tag="tp1", bufs=3)
                    nc.tensor.transpose(tp1[:],
                                        z_t[:, k * 128:(k + 1) * 128],
                                        ident[:])
                    zk = cp.tile([128, 128], F16 if AG16 else F32,
                                 name=f"zt_{t}_{k}", tag=f"zt_{t}_{k}")
                    nc.vector.tensor_copy(zk[:], tp1[:])
                    zt[t][k] = zk

            emit_traj_transposes(0)
            z0 = emit_proj_norm(0)
            emit_traj_transposes(1)   # PE fills the norm-0 wait
            emit_zt(0, z0)
            if mode != "prep":
                emit_ag(0)
            z1 = emit_proj_norm(1)
            emit_zt(1, z1)
            if mode != "prep":
                emit_ag(1)

            # preload the Exp table (ordered after both Sqrt ops)
            dummy_e = wp.tile([128, 1], F32, name="dummy_e", tag="dummy_e")
            nc.scalar.activation(dummy_e[:], z1[:, 0:1],
                                 mybir.ActivationFunctionType.Exp)

        if mode == "prep":
            res = wp.tile([1, 2], F32, name="res", tag="res")
            nc.vector.tensor_copy(res[:], zt[1][1][:1, 0:2])
            nc.sync.dma_start(out=out[:, :], in_=res[:])
            return

        emit_rhs_and_partner(0)
        emit_rhs_and_partner(1)

        with (
            tc.tile_pool(name="psAcc", bufs=1, space="PSUM") as psAcc,
            tc.tile_pool(name="psG", bufs=PSG_BUFS, space="PSUM") as psG,
        ):
            # ---- G = z_shard @ z_full^T (+mask), exp, row-sum ------
            # All half-0 groups first: they only need AllGather 0 and
            # overlap with AllGather 1.
            s_parts = []
            for mt in range(NT):
                sp_mt = wp.tile([128, 4], F32, name=f"sparts_{mt}",
                                tag=f"sparts_{mt}")
                s_parts.append(sp_mt)
            log_s = [None] * NT

            def emit_lse(mt):
                s_sum = wp.tile([128, 1], F32, name=f"ssum_{mt}",
                                tag=f"ssum_{mt}")
                nc.vector.tensor_reduce(s_sum[:], s_parts[mt][:],
                                        axis=mybir.AxisListType.X,
                                        op=mybir.AluOpType.add)
                ls = wp.tile([128, 1], F32, name=f"logs_{mt}",
                             tag=f"logs_{mt}")
                nc.scalar.activation(ls[:], s_sum[:],
                                     mybir.ActivationFunctionType.Ln)
                log_s[mt] = ls

            for h in range(NT):
                for mt in range(NT):
                    oh_v = onehot[mt].rearrange("p (r u j) -> p r u j",
                                                r=NCORES, u=2)
                    for q in range(2):
                        g = psG.tile([128, 512], F32, tag="g")
                        for k in range(KT):
                            nc.tensor.matmul(
                                g[:], lhsT=zt[mt][k][:], rhs=rhs[h][k][q][:],
                                start=(k == 0), stop=False)
                        nc.tensor.matmul(
                            g[:], lhsT=idb[:],
                            rhs=oh_v[:, 4 * q:4 * q + 4, h, :],
                            start=False, stop=True)
                        e_scr = sp.tile([128, 512], F32, tag="e", bufs=4)
                        nc.scalar.activation(
                            e_scr[:], g[:], mybir.ActivationFunctionType.Exp,
                            bias=neg_shift[:, :1],
                            accum_out=s_parts[mt][:, 2 * h + q:
                                                  2 * h + q + 1])
                    if LN_HOIST and h == NT - 1:
                        # this row tile's S is complete: log(S) overlaps the
                        # remaining exps instead of serializing in the tail
                        emit_lse(mt)

            # ---- labels: l[n] = sum_p z[p, n] * z_partner[p, n] ----
            l_ps = psAcc.tile([1, P], F32, tag="l")
            for mt in range(NT):
                for k in range(KT):
                    pp = wp.tile([128, 128], F32, name=f"pp_{mt}_{k}",
                                 tag=f"pp_{mt}_{k}")
                    nc.vector.tensor_tensor(pp[:], zt[mt][k][:],
                                            partner[mt][k][:],
                                            op=mybir.AluOpType.mult)
                    nc.tensor.matmul(
                        l_ps[:, mt * 128:(mt + 1) * 128],
                        lhsT=ones_col[:], rhs=pp[:],
                        start=(k == 0), stop=(k == KT - 1))

            if not LN_HOIST:
                for mt in range(NT):
                    emit_lse(mt)
            a_ps = psAcc.tile([1, 1], F32, tag="a")
            for mt in range(NT):
                nc.tensor.matmul(a_ps[:], lhsT=log_s[mt][:], rhs=ones_col[:],
                                 start=(mt == 0), stop=(mt == NT - 1))

            res = wp.tile([1, 2], F32, name="res", tag="res")
            nc.vector.tensor_copy(res[:, 0:1], a_ps[:])
            nc.vector.tensor_reduce(res[:, 1:2], l_ps[:],
                                    axis=mybir.AxisListType.X,
                                    op=mybir.AluOpType.add)
            nc.sync.dma_start(out=out[:, :], in_=res[:])


_NC_CACHE = {}


def _get_nc():
    if "nc" not in _NC_CACHE:
        _NC_CACHE["nc"] = build_nc()
    return _NC_CACHE["nc"]


def make_in_maps(representations, proj_w, proj_b, input_lengths):
    reps = np.ascontiguousarray(np.asarray(representations, dtype=np.float32))
    lengths = np.asarray(input_lengths).astype(np.int32)
    w = np.ascontiguousarray(np.asarray(proj_w, dtype=np.float32))
    b = np.ascontiguousarray(np.asarray(proj_b, dtype=np.float32))
    in_maps = []
    ar = np.arange(128, dtype=np.int32)
    for c in range(NCORES):
        partner = (c + NCORES // 2) % NCORES
        meta = np.zeros((128, 8), np.int32)
        for t in range(NT):
            meta[:, t] = lengths[c * SH + t * 128: c * SH + (t + 1) * 128]
            diag = (c * SH + t * 128 + ar).astype(np.float32)
            meta[:, 2 + t] = diag.view(np.int32)
        meta[:, 4] = partner * 128 + ar
        meta[:, 5] = partner * P + ar
        meta[:, 6] = partner * P + 128 + ar
        oh = np.zeros((128, NT * B2), ml_dtypes.bfloat16)
        for t in range(NT):
            oh[ar, t * B2 + c * SH + t * 128 + ar] = 1.0
        gm = np.zeros((128, 32), np.int32)
        p16 = ar % 16
        for t in range(NT):
            for s_col in range(8):
                n = t * 128 + 16 * s_col + p16
                gm[:, t * 16 + s_col] = n * S
                gm[:, t * 16 + 8 + s_col] = lengths[c * SH + n]
        in_maps.append({
            "reps": np.ascontiguousarray(
                reps[c * SH:(c + 1) * SH].reshape(SH * S, D)),
            "proj_w": w,
            "proj_b": b,
            "meta": meta,
            "gmeta": gm,
            "onehots": oh,
        })
    return in_maps


def combine_outputs(results):
    total = 0.0
    for r in results:
        a, l = np.asarray(r["out"], dtype=np.float64).ravel()
        total += a - l
    return np.float32(total / B2 + INV_T)


def kernel(representations, proj_w, proj_b, input_lengths):
    nc = _get_nc()
    in_maps = make_in_maps(representations, proj_w, proj_b, input_lengths)
    res = run_bass_kernel_spmd(nc, in_maps, core_ids=list(range(NCORES)))
    return np.asarray(combine_outputs(res.results), dtype=np.float32)


# revision 40
# speedup vs baseline: 1.0200x; 1.0200x over previous
"""Distributed NT-Xent contrastive loss kernel for Trainium2 (8 NeuronCores).

Strategy (data-parallel over batch, all-gather of projections):
  - Each core owns a 256-row shard of the 2048-sequence batch.
  - On device: gather last-valid-timestep rows (indirect DMA), project
    D=512 -> P=256 (fp32 matmuls, bias folded in as an extra K=1
    contraction row), row-normalize (cosine) with the 1/temperature
    scale folded in as sqrt(20) on each operand, transpose to [P, n]
    layout, AllGather the projections across the 8 cores (split into
    two 128-dim halves so the second collective overlaps the first
    half's similarity matmuls).
  - Each core computes its 256 rows of the full 2048x2048 logit matrix
    G = z_shard @ z_full^T. The own-diagonal mask (-1e30) is folded
    into each PSUM accumulation group as one extra bf16 matmul:
    (-1e30 * I) @ onehot, where onehot[i, j] = (j == diag_col_i) is
    built once per row-tile from an iota compare. Then a fused
    exp(x - 20) + row-sum runs on the scalar engine (20 is the
    self-similarity upper bound, so no max pass is needed), then
    log -> per-row logsumexp - 20.
  - The label logits (diag of sim12) are the elementwise product of the
    core's shard with its partner core's shard (indirect-gathered from
    the AllGather output), reduced over P via a ones-matmul.
  - Each core outputs [sum_rows log(S), sum_rows label]; the host sums
    the 8 partial pairs: loss = (sum A - sum L)/2048 + 20.
"""

import os
import sys

import ml_dtypes
import numpy as np

for _p in ("/root/.axon_site", "/root/.axon_site/_ro/trn_rl_repo",
           "/root/.axon_site/_ro/pypackages", "/opt/trn_rl_repo"):
    if os.path.isdir(_p) and _p not in sys.path:
        sys.path.append(_p)

import concourse.bacc as bacc
import concourse.bass as bass
import concourse.mybir as mybir
import concourse.tile as tile
from concourse.bass import IndirectOffsetOnAxis
from concourse.bass_utils import run_bass_kernel_spmd
from concourse.masks import make_identity

F32 = mybir.dt.float32
BF16 = mybir.dt.bfloat16
I32 = mybir.dt.int32
I16 = mybir.dt.int16

NCORES = 8
B2, S, D, P = 2048, 64, 512, 256
B = B2 // 2
SH = B2 // NCORES          # 256 rows per core
NT = SH // 128             # row tiles per core (2)
KT = P // 128              # contraction tiles over projection dim (2)
DT = D // 128              # tiles over representation dim (4)
NB = B2 // 512             # 512-wide column blocks of G (4)
INV_T = 20.0               # 1 / temperature
NEG_BIG = -1.0e30
F16 = mybir.dt.float16
# All-gather the projections in fp16: half the collective bytes and 4x
# faster similarity matmuls. Measured loss error vs the fp32 reference is
# 1.6e-06 relative (the fp16 rounding averages out over the 2048-row mean);
# set NTX_AG16=0 to fall back to full fp32 (6.3e-07).
AG16 = os.environ.get("NTX_AG16", "1") == "1"
# Measured same-process: psG=6 banks + hoisted log(S) is ~9 us SLOWER (the
# hoisted Ln between exp batches forces an ACT Exp-table reload, and a fully
# packed PSUM removes scheduler slack). Keep 4 banks and deferred Ln.
PSG_BUFS = 4
LN_HOIST = False


def build_nc(repeat=1):
    nc = bacc.Bacc("TRN2", target_bir_lowering=False, debug=False,
                   num_devices=NCORES, num_swdge_queues=2)

    reps = nc.dram_tensor("reps", [SH * S, D], F32, kind="ExternalInput")
    proj_w = nc.dram_tensor("proj_w", [P, D], F32, kind="ExternalInput")
    proj_b = nc.dram_tensor("proj_b", [P], F32, kind="ExternalInput")
    # meta int32 [128, 8]: cols 0,1 = lengths (t=0,1); 2,3 = diag col
    # (f32 bits, t=0,1); 4 = partner row index within a 128-row AG block.
    meta = nc.dram_tensor("meta", [128, 8], I32, kind="ExternalInput")
    # gmeta int32 [128, 32]: dma_gather index metadata in the 16-partition
    # wrap (idx i at [i%16, i//16], replicated over the 8 partition groups).
    # cols 0:16 = sequence base 64*n, cols 16:32 = lengths[n], n = 16*s+p%16.
    gmeta = nc.dram_tensor("gmeta", [128, 32], I32, kind="ExternalInput")
    # own-diagonal one-hot rows (bf16), used as the mask matmul rhs:
    # onehots[p, t, j] = (j == global row index of local row t*128+p).
    onehots = nc.dram_tensor("onehots", [128, NT * B2], BF16,
                             kind="ExternalInput")
    out = nc.dram_tensor("out", [1, 2], F32, kind="ExternalOutput")

    for _ in range(repeat):
        with tile.TileContext(nc) as tc:
            _body(tc, nc, reps, proj_w, proj_b, meta, gmeta, onehots, out)
    nc.compile()
    return nc


def _body(tc, nc, reps, proj_w, proj_b, meta, gmeta, onehots, out,
          mode="full"):
    with (
        tc.tile_pool(name="const", bufs=1) as cp,
        tc.tile_pool(name="work", bufs=1) as wp,
        tc.tile_pool(name="scratch", bufs=3) as sp,
        tc.tile_pool(name="dram", bufs=1, space="DRAM") as dp,
    ):
        # ---- constants / metadata ----------------------------------
        ident = cp.tile([128, 128], F32)
        make_identity(nc, ident)
        idb = cp.tile([128, 128], BF16)     # -1e30 * I for the mask matmul
        nc.vector.tensor_scalar_mul(idb[:], ident[:], NEG_BIG)
        ones_k1 = cp.tile([1, 128], F32)
        nc.gpsimd.memset(ones_k1[:], 1.0)
        ones_col = cp.tile([128, 1], F32)
        nc.gpsimd.memset(ones_col[:], 1.0)
        neg_shift = cp.tile([128, 1], F32)
        nc.gpsimd.memset(neg_shift[:], -INV_T)

        meta_sb = cp.tile([128, 8], I32)
        nc.sync.dma_start(out=meta_sb[:], in_=meta[:, :])
        b_sb = cp.tile([1, P], F32)
        nc.sync.dma_start(out=b_sb[:], in_=proj_b[None, :])
        w_sb = []
        for h in range(2):
            w_h = cp.tile([128, D], F32, name=f"w_{h}", tag=f"w_{h}")
            nc.sync.dma_start(out=w_h[:], in_=proj_w[h * 128:(h + 1) * 128, :])
            w_sb.append(w_h)

        if mode == "empty":
            res = wp.tile([1, 2], F32, name="res", tag="res")
            nc.vector.tensor_copy(res[:], meta_sb[:1, 0:2].bitcast(F32))
            nc.sync.dma_start(out=out[:, :], in_=res[:])
            return

        # own-diagonal one-hots (bf16), used as the mask matmul rhs
        oh_sb = cp.tile([128, NT, B2], BF16)
        nc.sync.dma_start(out=oh_sb[:],
                          in_=onehots.rearrange("p (t j) -> p t j", t=NT))
        onehot = [oh_sb[:, t, :] for t in range(NT)]

        if mode == "consts":
            res = wp.tile([1, 2], F32, name="res", tag="res")
            nc.vector.tensor_tensor(res[:], onehot[0][:1, 0:2],
                                    onehot[1][:1, 0:2],
                                    op=mybir.AluOpType.add)
            nc.sync.dma_start(out=out[:, :], in_=res[:])
            return

        # ---- gather last-valid-timestep rows (one dma_gather per
        # 128-row half, on separate SWDGE queues) ---------------------
        gmeta_sb = cp.tile([128, 32], I32)
        nc.sync.dma_start(out=gmeta_sb[:], in_=gmeta[:, :])
        traj = []
        for t in range(NT):
            base = gmeta_sb[:, t * 16:t * 16 + 8]
            lens = gmeta_sb[:, t * 16 + 8:t * 16 + 16]
            idx_t = wp.tile([128, 8], I32, name=f"idx_{t}", tag=f"idx_{t}")
            nc.vector.tensor_scalar(idx_t[:], lens, -1, 0,
                                    op0=mybir.AluOpType.add,
                                    op1=mybir.AluOpType.max)
            nc.vector.tensor_scalar(idx_t[:], idx_t[:], S - 1, None,
                                    op0=mybir.AluOpType.min)
            nc.vector.tensor_tensor(idx_t[:], idx_t[:], base,
                                    op=mybir.AluOpType.add)
            idxs16 = wp.tile([128, 8], I16, name=f"idxs16_{t}",
                             tag=f"idxs16_{t}")
            nc.vector.tensor_copy(idxs16[:], idx_t[:])
            gout_t = wp.tile([128, 1, D], F32, name=f"gout_{t}",
                             tag=f"gout_{t}")
            nc.gpsimd.dma_gather(
                out_ap=gout_t[:], in_ap=reps[:, :], idxs_ap=idxs16[:],
                num_idxs=128, num_idxs_reg=128, elem_size=D, queue_num=t)
            traj.append(gout_t[:, 0, :])

        if mode == "gather":
            res = wp.tile([1, 2], F32, name="res", tag="res")
            nc.vector.tensor_tensor(res[:], traj[0][:1, 0:2],
                                    traj[1][:1, 0:2],
                                    op=mybir.AluOpType.add)
            nc.sync.dma_start(out=out[:, :], in_=res[:])
            return

        zt = [[None] * KT for _ in range(NT)]   # zt[t][k]: [128p, 128n]
        ag_out = [None] * NT
        ag_in = [None] * NT

        def emit_ag(t):
            ag_in[t] = dp.tile([P, 128], F16 if AG16 else F32,
                               name=f"ag_in_{t}")
            for k in range(KT):
                nc.sync.dma_start(out=ag_in[t][k * 128:(k + 1) * 128, :],
                                  in_=zt[t][k][:])
            ag_out[t] = dp.tile(
                [NCORES * P, 128], F16 if AG16 else F32, name=f"ag_out_{t}",
                addr_space=("Local" if mode == "noag" else "Shared"))
            if mode == "noag":
                for r in range(NCORES):
                    nc.sync.dma_start(out=ag_out[t][r * P:(r + 1) * P, :],
                                      in_=ag_in[t][:, :])
            else:
                nc.gpsimd.collective_compute(
                    "AllGather", mybir.AluOpType.bypass,
                    replica_groups=[list(range(NCORES))],
                    ins=[ag_in[t].opt()], outs=[ag_out[t].opt()])

        rhs = [[None] * KT for _ in range(NT)]
        partner = [[None] * KT for _ in range(NT)]

        def emit_rhs_and_partner(h):
            ag_v = ag_out[h].rearrange("(r q) j -> q r j", r=NCORES)
            for k in range(KT):
                rt = wp.tile([128, NCORES, 128], F16 if AG16 else F32,
                             name=f"rhs_{h}_{k}", tag=f"rhs_{h}_{k}")
                nc.sync.dma_start(out=rt[:],
                                  in_=ag_v[k * 128:(k + 1) * 128, :, :])
                rhs[h][k] = rt
            # partner shard columns for the labels of own row-tile h
            for k in range(KT):
                pt = wp.tile([128, 128], F16 if AG16 else F32,
                             name=f"part_{h}_{k}", tag=f"part_{h}_{k}")
                pc = 5 + k
                nc.gpsimd.indirect_dma_start(
                    out=pt[:], out_offset=None, in_=ag_out[h][:, :],
                    in_offset=IndirectOffsetOnAxis(ap=meta_sb[:, pc:pc + 1],
                                                   axis=0))
                partner[h][k] = pt

        with tc.tile_pool(name="psA", bufs=1, space="PSUM") as psA:
            # w -> wT [d, p] (shared by both halves)
            wT = [cp.tile([128, P], F16 if AG16 else F32, name=f"wT_{d}",
                     tag=f"wT_{d}") for d in range(DT)]
            for d in range(DT):
                tpw = psA.tile([128, 256], F32, tag="tpw", bufs=2)
                for h in range(2):
                    nc.tensor.transpose(tpw[:, h * 128:(h + 1) * 128],
                                        w_sb[h][:, d * 128:(d + 1) * 128],
                                        ident[:])
                nc.vector.tensor_copy(wT[d][:], tpw[:])

            trajT = [[None] * DT for _ in range(NT)]

            def emit_traj_transposes(t):
                for d in range(DT):
                    tp1 = psA.tile([128, 128], F32, tag="tp1", bufs=3)
                    nc.tensor.transpose(tp1[:],
                                        traj[t][:, d * 128:(d + 1) * 128],
                                        ident[:])
                    tt = cp.tile([128, 128], F16 if AG16 else F32,
                                 name=f"trajT_{t}_{d}",
                                 tag=f"trajT_{t}_{d}")
                    nc.vector.tensor_copy(tt[:], tp1[:])
                    trajT[t][d] = tt

            def emit_proj_norm(t):
                proj_ps = psA.tile([128, P], F32, tag="proj", bufs=2)
                for d in range(DT):
                    nc.tensor.matmul(proj_ps[:], lhsT=trajT[t][d][:],
                                     rhs=wT[d][:], start=(d == 0), stop=False)
                nc.tensor.matmul(proj_ps[:], lhsT=ones_k1[:], rhs=b_sb[:],
                                 start=False, stop=True)
                proj_sb = wp.tile([128, P], F32, name=f"proj_{t}",
                                  tag=f"proj_{t}")
                nc.vector.tensor_copy(proj_sb[:], proj_ps[:])
                sq_scr = sp.tile([128, P], F32, tag="sq")
                nsq = wp.tile([128, 1], F32, name=f"nsq_{t}", tag=f"nsq_{t}")
                nc.vector.scalar_tensor_tensor(
                    sq_scr[:], in0=proj_sb[:], scalar=1.0, in1=proj_sb[:],
                    op0=mybir.AluOpType.mult, op1=mybir.AluOpType.mult,
                    accum_out=nsq[:])
                # inv = sqrt(INV_T / max(nsq, 1e-16))
                nc.vector.tensor_scalar_max(nsq[:], nsq[:], 1e-16)
                rcp = wp.tile([128, 1], F32, name=f"rcp_{t}", tag=f"rcp_{t}")
                nc.vector.reciprocal(rcp[:], nsq[:])
                nc.vector.tensor_scalar_mul(rcp[:], rcp[:], INV_T)
                inv = wp.tile([128, 1], F32, name=f"inv_{t}", tag=f"inv_{t}")
                nc.scalar.activation(inv[:], rcp[:],
                                     mybir.ActivationFunctionType.Sqrt)
                z_t = wp.tile([128, P], F32, name=f"z_{t}", tag=f"z_{t}")
                nc.vector.tensor_scalar_mul(z_t[:], proj_sb[:], inv[:, :1])
                return z_t

            def emit_zt(t, z_t):
                for k in range(KT):
                    tp1 = psA.tile([128, 128], F32, tag="tp1", bufs=3)
                    nc.tensor.transpose(tp1[:],
                                        z_t[:, k * 128:(k + 1) * 128],
                                        ident[:])
                    zk = cp.tile([128, 128], F16 if AG16 else F32,
                                 name=f"zt_{t}_{k}", tag=f"zt_{t}_{k}")
                    nc.vector.tensor_copy(zk[:], tp1[:])
                    zt[t][k] = zk

            emit_traj_transposes(0)
            z0 = emit_proj_norm(0)
            emit_traj_transposes(1)   # PE fills the norm-0 wait
            emit_zt(0, z0)
            if mode != "prep":
                emit_ag(0)
            z1 = emit_proj_norm(1)
            emit_zt(1, z1)
            if mode != "prep":
                emit_ag(1)

            # preload the Exp table (ordered after both Sqrt ops)
            dummy_e = wp.tile([128, 1], F32, name="dummy_e", tag="dummy_e")
            nc.scalar.activation(dummy_e[:], z1[:, 0:1],
                                 mybir.ActivationFunctionType.Exp)

        if mode == "prep":
            res = wp.tile([1, 2], F32, name="res", tag="res")
            nc.vector.tensor_copy(res[:], zt[1][1][:1, 0:2])
            nc.sync.dma_start(out=out[:, :], in_=res[:])
            return

        emit_rhs_and_partner(0)
        emit_rhs_and_partner(1)

        with (
            tc.tile_pool(name="psAcc", bufs=1, space="PSUM") as psAcc,
            tc.tile_pool(name="psG", bufs=PSG_BUFS, space="PSUM") as psG,
        ):
            # ---- G = z_shard @ z_full^T (+mask), exp, row-sum ------
            # All half-0 groups first: they only need AllGather 0 and
            # overlap with AllGather 1.
            s_parts = []
            for mt in range(NT):
                sp_mt = wp.tile([128, 4], F32, name=f"sparts_{mt}",
                                tag=f"sparts_{mt}")
                s_parts.append(sp_mt)
            log_s = [None] * NT

            def emit_lse(mt):
                s_sum = wp.tile([128, 1], F32, name=f"ssum_{mt}",
                                tag=f"ssum_{mt}")
                nc.vector.tensor_reduce(s_sum[:], s_parts[mt][:],
                                        axis=mybir.AxisListType.X,
                                        op=mybir.AluOpType.add)
                ls = wp.tile([128, 1], F32, name=f"logs_{mt}",
                             tag=f"logs_{mt}")
                nc.scalar.activation(ls[:], s_sum[:],
                                     mybir.ActivationFunctionType.Ln)
                log_s[mt] = ls

            for h in range(NT):
                for mt in range(NT):
                    oh_v = onehot[mt].rearrange("p (r u j) -> p r u j",
                                                r=NCORES, u=2)
                    for q in range(2):
                        g = psG.tile([128, 512], F32, tag="g")
                        for k in range(KT):
                            nc.tensor.matmul(
                                g[:], lhsT=zt[mt][k][:],
                                rhs=rhs[h][k][:, 4 * q:4 * q + 4, :],
                                start=(k == 0), stop=False)
                        nc.tensor.matmul(
                            g[:], lhsT=idb[:],
                            rhs=oh_v[:, 4 * q:4 * q + 4, h, :],
                            start=False, stop=True)
                        e_scr = sp.tile([128, 512], F32, tag="e", bufs=4)
                        nc.scalar.activation(
                            e_scr[:], g[:], mybir.ActivationFunctionType.Exp,
                            bias=neg_shift[:, :1],
                            accum_out=s_parts[mt][:, 2 * h + q:
                                                  2 * h + q + 1])
                    if LN_HOIST and h == NT - 1:
                        # this row tile's S is complete: log(S) overlaps the
                        # remaining exps instead of serializing in the tail
                        emit_lse(mt)

            # ---- labels: l[n] = sum_p z[p, n] * z_partner[p, n] ----
            l_ps = psAcc.tile([1, P], F32, tag="l")
            for mt in range(NT):
                for k in range(KT):
                    pp = wp.tile([128, 128], F32, name=f"pp_{mt}_{k}",
                                 tag=f"pp_{mt}_{k}")
                    nc.vector.tensor_tensor(pp[:], zt[mt][k][:],
                                            partner[mt][k][:],
                                            op=mybir.AluOpType.mult)
                    nc.tensor.matmul(
                        l_ps[:, mt * 128:(mt + 1) * 128],
                        lhsT=ones_col[:], rhs=pp[:],
                        start=(k == 0), stop=(k == KT - 1))

            if not LN_HOIST:
                for mt in range(NT):
                    emit_lse(mt)
            a_ps = psAcc.tile([1, 1], F32, tag="a")
            for mt in range(NT):
                nc.tensor.matmul(a_ps[:], lhsT=log_s[mt][:], rhs=ones_col[:],
                                 start=(mt == 0), stop=(mt == NT - 1))

            res = wp.tile([1, 2], F32, name="res", tag="res")
            nc.vector.tensor_copy(res[:, 0:1], a_ps[:])
            nc.vector.tensor_reduce(res[:, 1:2], l_ps[:],
                                    axis=mybir.AxisListType.X,
                                    op=mybir.AluOpType.add)
            nc.sync.dma_start(out=out[:, :], in_=res[:])


_NC_CACHE = {}


def _get_nc():
    if "nc" not in _NC_CACHE:
        _NC_CACHE["nc"] = build_nc()
    return _NC_CACHE["nc"]


def make_in_maps(representations, proj_w, proj_b, input_lengths):
    reps = np.ascontiguousarray(np.asarray(representations, dtype=np.float32))
    lengths = np.asarray(input_lengths).astype(np.int32)
    w = np.ascontiguousarray(np.asarray(proj_w, dtype=np.float32))
    b = np.ascontiguousarray(np.asarray(proj_b, dtype=np.float32))
    in_maps = []
    ar = np.arange(128, dtype=np.int32)
    for c in range(NCORES):
        partner = (c + NCORES // 2) % NCORES
        meta = np.zeros((128, 8), np.int32)
        for t in range(NT):
            meta[:, t] = lengths[c * SH + t * 128: c * SH + (t + 1) * 128]
            diag = (c * SH + t * 128 + ar).astype(np.float32)
            meta[:, 2 + t] = diag.view(np.int32)
        meta[:, 4] = partner * 128 + ar
        meta[:, 5] = partner * P + ar
        meta[:, 6] = partner * P + 128 + ar
        oh = np.zeros((128, NT * B2), ml_dtypes.bfloat16)
        for t in range(NT):
            oh[ar, t * B2 + c * SH + t * 128 + ar] = 1.0
        gm = np.zeros((128, 32), np.int32)
        p16 = ar % 16
        for t in range(NT):
            for s_col in range(8):
                n = t * 128 + 16 * s_col + p16
                gm[:, t * 16 + s_col] = n * S
                gm[:, t * 16 + 8 + s_col] = lengths[c * SH + n]
        in_maps.append({
            "reps": np.ascontiguousarray(
                reps[c * SH:(c + 1) * SH].reshape(SH * S, D)),
            "proj_w": w,
            "proj_b": b,
            "meta": meta,
            "gmeta": gm,
            "onehots": oh,
        })
    return in_maps


def combine_outputs(results):
    total = 0.0
    for r in results:
        a, l = np.asarray(r["out"], dtype=np.float64).ravel()
        total += a - l
    return np.float32(total / B2 + INV_T)


def kernel(representations, proj_w, proj_b, input_lengths):
    nc = _get_nc()
    in_maps = make_in_maps(representations, proj_w, proj_b, input_lengths)
    res = run_bass_kernel_spmd(nc, in_maps, core_ids=list(range(NCORES)))
    return np.asarray(combine_outputs(res.results), dtype=np.float32)
